# revision 1
# baseline (speedup 1.0000x reference)
"""CrossAssetGNN (GAT layer) Trainium2 kernel.

Strategy: edges sorted by destination on host; each of the 8 cores owns a
contiguous, 128-aligned destination-node range (edge-balanced), so no
cross-core reduction is needed. Per core:

  Phase 1 (dense): h8[n] = [h(n) (128 f32) | attn_src(n) (4 f32) | pad] 768B
  rows + att[n] = [attn_src, attn_dst] (8 f32) via PE matmuls of x^T against
  an extended weight matrix [W | W@a_src | W@a_dst].

  Phase 2 (per 128-dst-node window): dma_gather of h8 rows by edge source
  (descriptor-bound), attn_dst expanded per edge with a host-shipped
  transposed one-hot (bf16 hi/lo pair for near-f32 accuracy), per-edge
  coefficient exp(leakyrelu(asrc+adst)*w) on DVE/ACT, destination one-hot
  built on device by iota-compare, and a PSUM-accumulated matmul
  onehot^T @ [coeff*h | coeff] giving the fused numerator+denominator of the
  segment softmax (the global-max stabilization cancels mathematically up to
  the 1e-10 epsilon, ~1e-9 relative). Divide + store per window.

Self-contained: hardcodes all shapes from the problem spec.
"""

import math
import sys
import types
from contextlib import ExitStack

import numpy as np
import ml_dtypes

import concourse.bass as bass
import concourse.tile as tile
from concourse import bacc, mybir
from concourse import bass_utils

P = 128
N_NODES = 50000
N_EDGES = 1600000
IN_F = 128
OUT_F = 32
HEADS = 4
NEG_SLOPE = 0.2
NCORES = 8
NPAD = ((N_NODES + P - 1) // P) * P          # 50048
LOHI = 32768                                  # int16 index split
ROWF = 192                                    # h8 row: 128 h + 4 asrc + pad (768B)
RHSF = HEADS * OUT_F + HEADS                  # 132
X_CHUNK = 512                                 # phase-1 node chunk

_cache = {}


def _build_program(nwin, t_lo, t_hi):
    T = t_lo + t_hi
    nc = bacc.Bacc("TRN2", target_bir_lowering=False, debug=False,
                   enable_asserts=False, num_devices=NCORES, num_swdge_queues=4)
    f32, bf16, i16, i32 = (mybir.dt.float32, mybir.dt.bfloat16,
                           mybir.dt.int16, mybir.dt.int32)

    xT = nc.dram_tensor("xT", [P, NPAD], f32, kind="ExternalInput").ap()
    wc = nc.dram_tensor("wc", [P, 136], f32, kind="ExternalInput").ap()
    gidx = nc.dram_tensor("gidx", [nwin, P, T * 8], i16, kind="ExternalInput").ap()
    dstloc = nc.dram_tensor("dstloc", [nwin, P, T], bf16, kind="ExternalInput").ap()
    wgt = nc.dram_tensor("wgt", [nwin, P, T], f32, kind="ExternalInput").ap()
    onehT = nc.dram_tensor("onehT", [nwin, P, T * P], bf16, kind="ExternalInput").ap()
    dst0 = nc.dram_tensor("dst0", [1, 1], i32, kind="ExternalInput").ap()
    out = nc.dram_tensor("out", [nwin * P, IN_F], f32, kind="ExternalOutput").ap()

    h8 = nc.dram_tensor("h8", [NPAD, ROWF], f32, kind="Internal").ap()
    att = nc.dram_tensor("att", [NPAD + nwin * P, 8], f32, kind="Internal").ap()

    with tile.TileContext(nc) as tc:
        with ExitStack() as ctx:
            cst = ctx.enter_context(tc.tile_pool(name="cst", bufs=1))

            # ---- constants ----
            wc_sb = cst.tile([P, 136], f32)
            nc.sync.dma_start(wc_sb[:], wc[:])
            iota_i = cst.tile([P, P], i32)
            nc.gpsimd.iota(iota_i[:], pattern=[[1, P]], base=0, channel_multiplier=0)
            iota_f = cst.tile([P, P], f32)
            nc.vector.tensor_copy(iota_f[:], iota_i[:])
            # iota replicated across tiles, bf16, contiguous (fast compares)
            iota_big = cst.tile([P, T, P], bf16)
            nc.vector.tensor_copy(
                iota_big[:], iota_f[:].unsqueeze(1).to_broadcast([P, T, P]))
            dst0_sb = cst.tile([1, 1], i32)
            nc.sync.dma_start(dst0_sb[:], dst0[:])

            # ---- phase 1: h8 + att ----
            with ExitStack() as c1:
                p1 = c1.enter_context(tc.tile_pool(name="p1", bufs=3))
                ps1 = c1.enter_context(tc.tile_pool(name="ps1", bufs=4, space="PSUM"))
                nchunk = NPAD // X_CHUNK                       # 97
                rem = NPAD - nchunk * X_CHUNK                  # 50048 = 97*512+384
                chunks = [X_CHUNK] * nchunk + ([rem] if rem else [])
                base = 0
                for csz in chunks:
                    nj = csz // P
                    xc = p1.tile([P, X_CHUNK], f32, tag="xc")
                    nc.sync.dma_start(xc[:, :csz], xT[:, base:base + csz])
                    hrow = p1.tile([P, X_CHUNK // P, ROWF], f32, tag="hrow")
                    arow = p1.tile([P, X_CHUNK // P, 8], f32, tag="arow")
                    for j in range(nj):
                        ps = ps1.tile([P, 136], f32, space="PSUM")
                        nc.tensor.matmul(out=ps[:], lhsT=xc[:, j * P:(j + 1) * P],
                                         rhs=wc_sb[:], start=True, stop=True)
                        nc.scalar.copy(hrow[:, j, 0:136], ps[:])
                        nc.scalar.copy(arow[:, j, :], ps[:, 128:136])
                    # h8 rows [base .. base+csz): row (base + j*128 + p)
                    nc.sync.dma_start(
                        h8[base:base + csz, :].rearrange("(j p) c -> p j c", p=P),
                        hrow[:, :nj, :])
                    nc.sync.dma_start(
                        att[base:base + csz, :].rearrange("(j p) c -> p j c", p=P),
                        arow[:, :nj, :])
                    base += csz
                # zero the att overhang (windows past the core's range)
                zt = p1.tile([P, nwin, 8], f32, tag="zt")
                nc.vector.memset(zt[:], 0.0)
                nc.sync.dma_start(
                    att[NPAD:NPAD + nwin * P, :].rearrange("(w p) c -> p w c", p=P),
                    zt[:])

            # ---- per-core attn_dst windows (dynamic offset by dst0) ----
            dst0v = nc.values_load(dst0_sb[0:1, 0:1])
            attw = cst.tile([P, nwin, 8], f32)
            nc.sync.dma_start(
                attw[:],
                att[bass.ds(dst0v, nwin * P), :].rearrange("(w p) c -> p w c", p=P))
            # bf16 hi/lo split of attn_dst for exact-ish expansion matmuls
            att_hi = cst.tile([P, nwin, 4], bf16)
            nc.vector.tensor_copy(att_hi[:], attw[:, :, 4:8])
            att_hif = cst.tile([P, nwin, 4], f32)
            nc.vector.tensor_copy(att_hif[:], att_hi[:])
            att_lo = cst.tile([P, nwin, 4], bf16)
            nc.vector.tensor_sub(att_lo[:], attw[:, :, 4:8], att_hif[:])
            attw8 = cst.tile([P, nwin, 8], bf16)
            nc.vector.tensor_copy(attw8[:, :, 0:4], att_hi[:])
            nc.vector.tensor_copy(attw8[:, :, 4:8], att_lo[:])

            # ---- phase 2 ----
            p2 = ctx.enter_context(tc.tile_pool(name="p2", bufs=2))
            pe3 = ctx.enter_context(tc.tile_pool(name="pe3", bufs=2))
            gp = ctx.enter_context(tc.tile_pool(name="gp", bufs=2))
            ps_o = ctx.enter_context(tc.tile_pool(name="ps_o", bufs=2, space="PSUM"))
            ps_a = ctx.enter_context(tc.tile_pool(name="ps_a", bufs=2, space="PSUM"))

            h8_lo = h8[0:LOHI, :]
            h8_hi = h8[LOHI:NPAD, :]

            for w in range(nwin):
                gi = p2.tile([P, T * 8], i16, tag="gi")
                nc.sync.dma_start(gi[:], gidx[w])
                G = gp.tile([P, T, ROWF], f32, tag="G")
                nc.gpsimd.dma_gather(
                    G[:, 0:t_lo, :], h8_lo, gi[:, 0:t_lo * 8],
                    t_lo * P, t_lo * P, ROWF,
                    single_packet=False, queue_num=(2 * w) % 4)
                if t_hi:
                    nc.gpsimd.dma_gather(
                        G[:, t_lo:T, :], h8_hi, gi[:, t_lo * 8:T * 8],
                        t_hi * P, t_hi * P, ROWF,
                        single_packet=False, queue_num=(2 * w + 1) % 4)

                # attn_dst per edge slot: onehotT^T @ [att_hi | att_lo]
                ohT = pe3.tile([P, T * P], bf16, tag="ohT")
                nc.sync.dma_start(ohT[:], onehT[w])
                aps = ps_a.tile([P, T * 8], f32, space="PSUM")
                for t in range(T):
                    nc.tensor.matmul(out=aps[:, t * 8:(t + 1) * 8],
                                     lhsT=ohT[:, t * P:(t + 1) * P],
                                     rhs=attw8[:, w, :], start=True, stop=True)
                apsv = aps[:].rearrange("p (t c) -> p t c", c=8)

                dl = p2.tile([P, T], bf16, tag="dl")
                nc.sync.dma_start(dl[:], dstloc[w])
                wg = p2.tile([P, T], f32, tag="wg")
                nc.sync.dma_start(wg[:], wgt[w])

                # destination one-hot (edge-partition orientation), on device
                oh = pe3.tile([P, T, P], bf16, tag="oh")
                nc.vector.tensor_tensor(
                    out=oh[:],
                    in0=dl[:].unsqueeze(2).to_broadcast([P, T, P]),
                    in1=iota_big[:],
                    op=mybir.AluOpType.is_equal)

                # coeff = exp(leakyrelu(asrc + adst) * w)
                lg = p2.tile([P, T, 4], f32, tag="lg")
                nc.vector.tensor_add(lg[:], G[:, :, 128:132], apsv[:, :, 0:4])
                nc.vector.tensor_add(lg[:], lg[:], apsv[:, :, 4:8])
                lk = p2.tile([P, T, 4], f32, tag="lk")
                nc.vector.scalar_tensor_tensor(
                    out=lk[:], in0=lg[:], scalar=NEG_SLOPE, in1=lg[:],
                    op0=mybir.AluOpType.mult, op1=mybir.AluOpType.max)
                nc.vector.tensor_tensor(
                    out=lk[:], in0=lk[:],
                    in1=wg[:].unsqueeze(2).to_broadcast([P, T, 4]),
                    op=mybir.AluOpType.mult)
                cf = p2.tile([P, T, 4], f32, tag="cf")
                nc.scalar.activation(cf[:], lk[:], mybir.ActivationFunctionType.Exp)

                # rhs = [coeff*h | coeff]  (bf16 for fast matmul)
                rhs = pe3.tile([P, T, RHSF], bf16, tag="rhs")
                for h in range(HEADS):
                    nc.vector.tensor_tensor(
                        out=rhs[:, :, h * OUT_F:(h + 1) * OUT_F],
                        in0=G[:, :, h * OUT_F:(h + 1) * OUT_F],
                        in1=cf[:, :, h].unsqueeze(2).to_broadcast([P, T, OUT_F]),
                        op=mybir.AluOpType.mult)
                nc.vector.tensor_copy(rhs[:, :, 128:132], cf[:])

                ops = ps_o.tile([P, RHSF], f32, space="PSUM")
                for t in range(T):
                    nc.tensor.matmul(out=ops[:], lhsT=oh[:, t, :], rhs=rhs[:, t, :],
                                     start=(t == 0), stop=(t == T - 1))

                den = p2.tile([P, 4], f32, tag="den")
                nc.vector.tensor_scalar_add(den[:], ops[:, 128:132], 1e-10)
                rec = p2.tile([P, 4], f32, tag="rec")
                nc.vector.reciprocal(rec[:], den[:])
                ow = p2.tile([P, IN_F], f32, tag="ow")
                for h in range(HEADS):
                    nc.scalar.mul(ow[:, h * OUT_F:(h + 1) * OUT_F],
                                  ops[:, h * OUT_F:(h + 1) * OUT_F],
                                  rec[:, h:h + 1])
                nc.sync.dma_start(out[w * P:(w + 1) * P, :], ow[:])

    nc.compile()
    return nc


def _prep(x, edge_index, edge_weight, W, a_src, a_dst):
    x = np.asarray(x, np.float32)
    src = np.asarray(edge_index[0], np.int64)
    dst = np.asarray(edge_index[1], np.int64)
    ew = np.asarray(edge_weight, np.float32)
    W = np.asarray(W, np.float32)
    a_src = np.asarray(a_src, np.float32)[..., 0]
    a_dst = np.asarray(a_dst, np.float32)[..., 0]

    # extended weights: [W concat | W@a_src | W@a_dst]  -> [128, 136]
    wc = np.zeros((IN_F, 136), np.float32)
    wc[:, 0:128] = W.transpose(1, 0, 2).reshape(IN_F, HEADS * OUT_F)
    wc[:, 128:132] = np.einsum('hio,ho->ih', W, a_src)
    wc[:, 132:136] = np.einsum('hio,ho->ih', W, a_dst)

    xTp = np.zeros((IN_F, NPAD), np.float32)
    xTp[:, :N_NODES] = np.ascontiguousarray(x.T)

    order0 = np.argsort(dst, kind="stable")
    dsts = dst[order0]
    srcs = src[order0]
    ews = ew[order0]

    # core cuts: balanced by edges, aligned to 128-node boundaries
    bounds = [0]
    for c in range(1, NCORES):
        node = int(dsts[(N_EDGES * c) // NCORES])
        node = int(round(node / P)) * P
        node = min(max(node, bounds[-1] + P), NPAD - (NCORES - c) * P)
        bounds.append(node)
    bounds.append(NPAD)
    estart = np.searchsorted(dsts, bounds)
    nwin = max(
        (bounds[c + 1] - bounds[c]) // P for c in range(NCORES))

    # first pass: per-(core,window,class) counts to fix T_LO/T_HI globally
    per_core = []
    max_lo = max_hi = 0
    for c in range(NCORES):
        sl = slice(estart[c], estart[c + 1])
        s_c, d_c, w_c = srcs[sl], dsts[sl], ews[sl]
        wid = (d_c - bounds[c]) >> 7
        cls = (s_c >= LOHI).astype(np.int64)
        o2 = np.lexsort((cls, wid))
        s_c, d_c, w_c, wid, cls = s_c[o2], d_c[o2], w_c[o2], wid[o2], cls[o2]
        g = wid * 2 + cls
        cnt = np.bincount(g, minlength=nwin * 2)
        if len(cnt):
            max_lo = max(max_lo, int(cnt[0::2].max()))
            max_hi = max(max_hi, int(cnt[1::2].max()))
        per_core.append((s_c, d_c, w_c, wid, cls, g, cnt))
    t_lo = max(1, math.ceil(max_lo / P))
    t_hi = math.ceil(max_hi / P)
    T = t_lo + t_hi

    in_maps = []
    for c in range(NCORES):
        s_c, d_c, w_c, wid, cls, g, cnt = per_core[c]
        starts = np.zeros(nwin * 2, np.int64)
        np.cumsum(cnt[:-1], out=starts[1:])
        r = np.arange(len(g)) - starts[g]
        slot = np.where(cls == 1, t_lo * P, 0) + r
        pp = slot % P
        tt = slot // P

        gidx = np.zeros((nwin, 16, T * 8), np.int16)
        col = r // 16 + np.where(cls == 1, t_lo * 8, 0)
        gidx[wid, r % 16, col] = (s_c - cls * LOHI).astype(np.int16)
        gidx = np.tile(gidx, (1, 8, 1))

        dstloc = np.full((nwin, P, T), -1.0, ml_dtypes.bfloat16)
        dloc = d_c - bounds[c] - wid * P
        dstloc[wid, pp, tt] = dloc.astype(np.float32)  # cast below

        wgt = np.zeros((nwin, P, T), np.float32)
        wgt[wid, pp, tt] = w_c

        onehT = np.zeros((nwin, P, T * P), ml_dtypes.bfloat16)
        onehT[wid, dloc, slot] = 1.0

        in_maps.append({
            "xT": xTp, "wc": wc, "gidx": gidx, "dstloc": dstloc,
            "wgt": wgt, "onehT": onehT,
            "dst0": np.array([[bounds[c]]], np.int32),
        })
    return in_maps, bounds, nwin, t_lo, t_hi


def kernel(x, edge_index, edge_weight, W, a_src, a_dst):
    in_maps, bounds, nwin, t_lo, t_hi = _prep(
        x, edge_index, edge_weight, W, a_src, a_dst)
    key = (nwin, t_lo, t_hi)
    if key not in _cache:
        _cache[key] = _build_program(nwin, t_lo, t_hi)
    nc = _cache[key]
    res = bass_utils.run_bass_kernel_spmd(
        nc, in_maps, core_ids=list(range(NCORES)),
        trace=bool(__import__("os").environ.get("GNN_TRACE")))
    out = np.empty((N_NODES, IN_F), np.float32)
    for c in range(NCORES):
        lo, hi = bounds[c], min(bounds[c + 1], N_NODES)
        if hi > lo:
            out[lo:hi] = res.results[c]["out"][0:hi - lo]
    kernel.last_exec_time_ns = res.exec_time_ns
    return out

